# revision 17
# baseline (speedup 1.0000x reference)
"""FourierBlock kernel for 8 TRN2 NeuronCores (v4).

Math: the reference keeps only the first 64 rfft modes, so the whole op is
    out[b] = CS2 @ Y[b],  Y = combine(P),  P = products(X2, W),  X2 = F2 @ q[b]
with F2 [128,4096] = [cos; -sin] forward-DFT rows, P the four uncombined
product groups (XrWr, XiWr, XrWi, XiWi), `combine` a [48,16] +-1 matmul
folding them into Yr = XrWr - XiWi / Yi = XrWi + XiWr, and CS2 [128, L] the
inverse-DFT rows (2/L scaling, DC halved).

Sharding: core c owns batch c for steps 1/3 (data parallel) and modes
[8c, 8c+8) for step 2 (tensor parallel over modes -> W is read exactly once
across the chip).  Two AllToAlls exchange the small X2 / Y tensors.

v4 changes vs v3 (trace-driven):
- Dummy 1KB AllToAll fired at t=0: the FIRST collective after load pays a
  40-100us ncfw arming cost (doorbell -> ALGO_MESH_BEGIN); absorbing it in a
  dummy overlaps the arming with the q/W prologue, so the real A2A1 starts
  within ~1us of its doorbell like A2A2 always did.
- Host-side contiguous layouts for q and out (no strided 1KB descriptors);
  out is un-shuffled on the host.
- Three DMA lanes: big streams (fmat halves, q, W, out) split across the two
  HWDGE rings (sync/scalar); small latency-critical transfers (b1in bounce,
  b2in staging, pg loads) on the idle GpSimd SWDGE ring so they never queue
  behind a 4MB W stream.
- xm transpose via 4 DMA-transposes (HWDGE xbar) directly b1out->xt_sb,
  removing the PE transposes + DVE copies and the xm_sb intermediate.
- Step 2 copies: one [48,512] PSUM->SBUF copy per mode (cmb has zero rows in
  the 16-31 band; the 3 PSUM bufs are memset once so the dead band is 0),
  alternating DVE/ACT, with the combine-output copy on the other engine.
- A2A2 split in two mode-halves: the first exchange flies while the second
  half's products run, so only one ~6us mesh is exposed.
- WARM2 enlarged to bridge the A2A2b window so step 3 runs at 2.4 GHz.
"""

import numpy as np

B, L, D, M = 8, 4096, 512, 64
NCORES = 8
T = M // NCORES  # local modes per core
TH = T // 2  # modes per A2A2 half

WARM1 = 48  # N=128 filler matmuls covering the A2A1 window
WARM2 = 150  # N=128 fillers bridging A2A2b + pg load before step 3


def _constants():
    import ml_dtypes

    bf16 = ml_dtypes.bfloat16
    k = np.arange(M)
    l = np.arange(L)
    ang = 2 * np.pi * np.outer(k, l) / L  # [M, L]
    # F2 row order (s, a, t): partition p = s*16 + a*8 + t holds
    # cos (a=0) / -sin (a=1) of mode k = 8s + t, so x2's partition layout
    # already equals the AllToAll bounce layout [s][a, t] (straight DMA).
    F2 = np.stack([np.cos(ang), -np.sin(ang)], axis=0)  # [2, M, L]
    F2 = F2.reshape(2, NCORES, T, L).transpose(1, 0, 2, 3).reshape(128, L)
    # lhsT chunks, p-major for contiguous DMA: fmat[p, n, m] = F2[m, n*128+p]
    fmat = np.ascontiguousarray(
        F2.T.reshape(32, 128, 128).transpose(1, 0, 2), dtype=bf16
    )  # [128, 32, 128] bf16

    # Inverse-DFT rows, K order (s, c, t) = mode 8s+t, c = Re/Im part of Y.
    ck = np.where(k == 0, 1.0, 2.0) / L
    ang2 = 2 * np.pi * np.outer(l, k) / L  # [L, M]
    C = (ck * np.cos(ang2)).reshape(L, NCORES, T)  # coeff for Yr
    S = (-(2.0 / L) * np.sin(ang2)).reshape(L, NCORES, T)  # coeff for Yi
    CS2 = np.empty((L, NCORES, 2, T))
    CS2[:, :, 0] = C
    CS2[:, :, 1] = S
    # K-row order (h, s, c, th) with t = h*TH + th, so each A2A2 half's Y
    # lands in a contiguous 64-partition range of pg.
    CS2 = CS2.reshape(L, NCORES, 2, 2, TH).transpose(0, 3, 1, 2, 4)
    cmat = np.ascontiguousarray(CS2.reshape(L, 128).T.astype(bf16))  # [128, L]

    # Producer-side combine: po rows (j, a) in band g2*32 -> Y rows (j, c).
    # Yr = XrWr - XiWi;  Yi = XrWi + XiWr.
    G = np.zeros((2, 2, 2))  # [c, a, g2]
    G[0, 0, 0] = 1.0
    G[0, 1, 1] = -1.0
    G[1, 0, 1] = 1.0
    G[1, 1, 0] = 1.0
    c48 = np.zeros((48, 16))
    for j in range(8):
        for a in range(2):
            for g2 in range(2):
                for c in range(2):
                    c48[g2 * 32 + j * 2 + a, j * 2 + c] = G[c, a, g2]
    c48 = np.ascontiguousarray(c48, dtype=bf16)
    return fmat, cmat, c48


def build_nc(reps=1, debug=False):
    import concourse.bacc as bacc
    import concourse.mybir as mybir
    import concourse.tile as tile

    f32 = mybir.dt.float32
    bf16 = mybir.dt.bfloat16
    nc = bacc.Bacc("TRN2", target_bir_lowering=False, num_devices=NCORES)

    # q pre-arranged on host: qb[p, n, d] = q[c][n*128 + p, d]
    qb = nc.dram_tensor("qb", [128, 32, D], bf16, kind="ExternalInput")
    # W pre-arranged on host: w[g2][p, t, dc, e] = W_g2[dc*128+p, e, 8c+t], bf16
    wr = nc.dram_tensor("wr", [128, T, 4, 512], bf16, kind="ExternalInput")
    wi = nc.dram_tensor("wi", [128, T, 4, 512], bf16, kind="ExternalInput")
    # out[p, g, i, e] = out_full[g*512 + i*128 + p, e]  (host un-shuffles)
    out = nc.dram_tensor("out", [128, 8, 4, D], bf16, kind="ExternalOutput")

    fmat_d = nc.dram_tensor("fmat", [128, 32, 128], bf16, kind="ExternalInput")
    cmat_d = nc.dram_tensor("cmat", [128, L], bf16, kind="ExternalInput")
    cmb_d = nc.dram_tensor("cmb", [48, 16], bf16, kind="ExternalInput")
    ident_d = nc.dram_tensor("ident", [128, 128], bf16, kind="ExternalInput")
    if debug:
        dbg_x2 = nc.dram_tensor("dbg_x2", [128, 512], bf16, kind="ExternalOutput")
        dbg_xt = nc.dram_tensor("dbg_xt", [128, 512], bf16, kind="ExternalOutput")
        dbg_st = nc.dram_tensor("dbg_st", [48, T * 512], bf16, kind="ExternalOutput")
        dbg_y = nc.dram_tensor("dbg_y", [16, T * 512], bf16, kind="ExternalOutput")
        dbg_pg = nc.dram_tensor("dbg_pg", [128, 512], bf16, kind="ExternalOutput")

    RG = [list(range(NCORES))]

    from concourse.tile_rust import add_dep_helper

    with tile.TileContext(nc) as tc:
        with (
            tc.tile_pool(name="constp", bufs=1) as constp,
            tc.tile_pool(name="qpool", bufs=5) as qpool,
            tc.tile_pool(name="wpool", bufs=1) as wpool,
            tc.tile_pool(name="misc", bufs=1) as misc,
            tc.tile_pool(name="outp", bufs=8) as outp,
            tc.tile_pool(name="pacc", bufs=4, space="PSUM") as pacc,
            tc.tile_pool(name="ptp", bufs=1, space="PSUM") as ptp,
            tc.tile_pool(name="po", bufs=3, space="PSUM") as po,
            tc.tile_pool(name="dram", bufs=1, space="DRAM") as dram,
        ):
          for rep in range(reps):
            # ---- dummy collective: absorbs the one-time ncfw arming cost
            # (40-100us doorbell->mesh-begin on the first collective) while
            # the prologue streams q/W.  Payload is 1KB of uninitialized
            # DRAM; the output is never read.
            d_in = dram.tile([NCORES, 64], bf16, name="d_in")
            d_out = dram.tile([NCORES, 64], bf16, name="d_out")
            nc.gpsimd.collective_compute(
                "AllToAll",
                mybir.AluOpType.bypass,
                replica_groups=RG,
                ins=[d_in.opt()],
                outs=[d_out.opt()],
            )

            # ---- memset the 3 product-PSUM bufs once: the 16-31 partition
            # band is never written by the product matmuls but IS read by the
            # single [48,512] staging copy; cmb zeros it in the combine, but
            # NaN * 0 = NaN, so it must hold finite data.
            po_init = []
            for i in range(3):
                pz = po.tile([48, 512], f32, name="po_t", tag="o")
                nc.vector.memset(pz[:], 0.0)
                po_init.append(pz)

            # ---- constants + q + W on the two HWDGE rings.
            # sync ring:   fmat half 0, q chunks 0/2/4/6, wr (2x2MB), xt
            #              transposes, out even quads
            # scalar ring: fmat half 1, q chunks 1/3/5/7, ident/cmb, wi
            #              (2x2MB), cmat, out odd quads
            fmat_sb = constp.tile([128, 32 * 128], bf16, name="fmat_sb")
            fmat_v = fmat_sb[:].rearrange("p (n m) -> p n m", n=32)
            nc.sync.dma_start(out=fmat_v[:, 0:16], in_=fmat_d[:, 0:16])
            q_tiles = []
            q_dmas = []
            for lo in range(8):
                qt = qpool.tile([128, 4 * 512], bf16, name="qt", tag="qt")
                qeng = nc.sync if lo % 2 == 0 else nc.scalar
                q_dmas.append(qeng.dma_start(
                    out=qt[:].rearrange("p (n d) -> p n d", n=4),
                    in_=qb[:, lo * 4 : (lo + 1) * 4],
                ))
                q_tiles.append(qt)
                if lo == 0:
                    # fmat half 1 early on scalar: chunks 16-31 need it
                    nc.scalar.dma_start(
                        out=fmat_v[:, 16:32], in_=fmat_d[:, 16:32]
                    )
            cmb_sb = constp.tile([48, 16], bf16, name="cmb_sb")
            nc.scalar.dma_start(out=cmb_sb[:], in_=cmb_d[:])
            ident_sb = constp.tile([128, 128], bf16, name="ident_sb")
            nc.scalar.dma_start(out=ident_sb[:], in_=ident_d[:])

            # ---- step 1 (bf16): X2 = F2 @ qb -> [128 (s,a,t), 512 d]
            x2ps = pacc.tile([128, 512], f32, name="x2ps", tag="acc")
            for lo in range(8):
                qt = q_tiles[lo]
                for li in range(4):
                    gl = lo * 4 + li
                    nc.tensor.matmul(
                        x2ps[:],
                        lhsT=fmat_sb[:, gl * 128 : (gl + 1) * 128],
                        rhs=qt[:, li * 512 : (li + 1) * 512],
                        start=(gl == 0),
                        stop=(gl == 31),
                    )

            # ---- W stream: 2x2MB per tensor per ring.  The first chunk of
            # each tensor is gated on the last q chunks: ring-FIFO only
            # orders descriptor GENERATION, the SDMA engines drain all
            # queued packets round-robin, so un-gated W data would steal
            # half of q's HBM bandwidth and starve step 1.
            w_sb = []
            for g2 in range(2):
                wt = wpool.tile([128, T * 4 * 512], bf16, name=f"w{g2}")
                w_sb.append(wt)
            for g2, wsrc in enumerate((wr, wi)):
                weng = nc.sync if g2 == 0 else nc.scalar
                for h in range(2):
                    wdma = weng.dma_start(
                        out=w_sb[g2][:, h * 8192 : (h + 1) * 8192].rearrange(
                            "p (t n e) -> p t n e", t=4, n=4
                        ),
                        in_=wsrc[:, h * 4 : (h + 1) * 4],
                    )
                    if h == 0:
                        for qd in q_dmas[-2:]:
                            add_dep_helper(
                                wdma.ins, qd.ins, sync=True,
                                reason="q keeps full HBM bandwidth",
                            )
            cmat_sb = constp.tile([128, L], bf16, name="cmat_sb")
            nc.scalar.dma_start(out=cmat_sb[:], in_=cmat_d[:])

            # ---- x2 PSUM -> SBUF (split DVE/ACT), bounce on SWDGE ring
            x2sb = misc.tile([128, 512], bf16, name="x2sb")
            nc.vector.tensor_copy(x2sb[:, 0:256], x2ps[:, 0:256])
            nc.scalar.copy(x2sb[:, 256:512], x2ps[:, 256:512])

            b1in = dram.tile([NCORES, 2, T, D], bf16, name="b1in")
            nc.gpsimd.dma_start(
                out=b1in[:].rearrange("s a t d -> (s a t) d"), in_=x2sb[:]
            )
            # Keep the PE clock warm through a short A2A1 stall (HAM
            # re-throttles to 1.2 GHz after ~3.4us idle).
            warm1 = pacc.tile([128, 512], f32, name="warm1", tag="acc")
            for i in range(WARM1):
                nc.tensor.matmul(
                    warm1[:, 0:128], lhsT=x2sb[:, 0:128], rhs=x2sb[:, 0:128],
                    start=(i == 0), stop=(i == WARM1 - 1),
                )
            b1out = dram.tile([NCORES, 2, T, D], bf16, name="b1out")
            nc.gpsimd.collective_compute(
                "AllToAll",
                mybir.AluOpType.bypass,
                replica_groups=RG,
                ins=[b1in.opt()],
                outs=[b1out.opt()],
            )

            # ---- load Xm [128 (j,a,t), 512 d] on the SWDGE ring, transpose
            # on the PE (re-warms it) into xt_sb[p=d|dc, (j a t)].
            xm_sb = misc.tile([128, 512], bf16, name="xm_sb")
            nc.gpsimd.dma_start(
                out=xm_sb[:], in_=b1out[:].rearrange("j a t d -> (j a t) d")
            )
            xt_sb = misc.tile([128, 512], bf16, name="xt_sb")
            for dc in range(4):
                tp = ptp.tile([128, 128], bf16, name="tp", tag="tp")
                nc.tensor.transpose(
                    tp[:], xm_sb[:, dc * 128 : (dc + 1) * 128], ident_sb[:]
                )
                cp = nc.vector.tensor_copy if dc % 2 else nc.scalar.copy
                cp(xt_sb[:, dc * 128 : (dc + 1) * 128], tp[:])

            # ---- step 2 (bf16) + producer-side combine: per t, both g2
            # product groups land in one [48, 512] PSUM tile (bands 0-15 /
            # 32-47), bounce through SBUF, and a [48,16] +-1 matmul folds
            # them into Y[(j, c)].  Modes are processed in two halves with
            # an AllToAll per half so the first exchange hides under the
            # second half's products.
            xt_v = xt_sb[:].rearrange("p (dc m t) -> p dc m t", dc=4, m=16, t=T)
            st_big = misc.tile([48, T * 512], bf16, name="st_big")
            b2in = [
                dram.tile([NCORES, 2, TH, D], bf16, name=f"b2in{h}")
                for h in range(2)
            ]
            b2out = [
                dram.tile([NCORES, 2, TH, D], bf16, name=f"b2out{h}")
                for h in range(2)
            ]
            ystage = misc.tile([16, T * 512], bf16, name="ystage")

            def emit_products(t):
                po_t = po.tile([48, 512], f32, name="po_t", tag="o")
                for g2 in range(2):
                    for dc in range(4):
                        nc.tensor.matmul(
                            po_t[g2 * 32 : g2 * 32 + 16, :],
                            lhsT=xt_v[:, dc, :, t],
                            rhs=w_sb[g2][
                                :, (t * 4 + dc) * 512 : (t * 4 + dc + 1) * 512
                            ],
                            start=(dc == 0),
                            stop=(dc == 3),
                        )
                sl = st_big[:, t * 512 : (t + 1) * 512]
                cp = nc.vector.tensor_copy if t % 2 else nc.scalar.copy
                cp(sl, po_t[:])
                return sl

            def emit_combine(t, sl):
                yt = po.tile([16, 512], f32, name="yt", tag="o")
                nc.tensor.matmul(
                    yt[:], lhsT=cmb_sb[:], rhs=sl, start=True, stop=True
                )
                ysl = ystage[:, t * 512 : (t + 1) * 512]
                cp = nc.scalar.copy if t % 2 else nc.vector.tensor_copy
                cp(ysl, yt[:])
                h, th = divmod(t, TH)
                nc.gpsimd.dma_start(
                    out=b2in[h][:, :, th].rearrange("j c e -> (j c) e"),
                    in_=ysl,
                )

            # Lag the combine one mode behind the products so the PE never
            # waits on the staging copies; fire each A2A2 half as soon as
            # its last mode is staged.
            slots = {}
            for t in range(T):
                slots[t] = emit_products(t)
                if t >= 1:
                    emit_combine(t - 1, slots[t - 1])
                if t - 1 == TH - 1:
                    nc.gpsimd.collective_compute(
                        "AllToAll",
                        mybir.AluOpType.bypass,
                        replica_groups=RG,
                        ins=[b2in[0].opt()],
                        outs=[b2out[0].opt()],
                    )
            emit_combine(T - 1, slots[T - 1])
            nc.gpsimd.collective_compute(
                "AllToAll",
                mybir.AluOpType.bypass,
                replica_groups=RG,
                ins=[b2in[1].opt()],
                outs=[b2out[1].opt()],
            )

            warm2 = pacc.tile([128, 512], f32, name="warm2", tag="acc")
            for i in range(WARM2):
                nc.tensor.matmul(
                    warm2[:, 0:128],
                    lhsT=ystage[:, 0:128],
                    rhs=ystage[:, 0:128],
                    start=(i == 0), stop=(i == WARM2 - 1),
                )

            # ---- load Y [128 (h,s,c,th), 512]: one SWDGE DMA per half into
            # a contiguous 64-partition range (cmat's K order matches).
            pg = misc.tile([128, 512], bf16, name="pg")
            for h in range(2):
                nc.gpsimd.dma_start(
                    out=pg[h * 64 : (h + 1) * 64],
                    in_=b2out[h][:].rearrange("s c t e -> (s c t) e"),
                )

            if debug:
                nc.sync.dma_start(out=dbg_x2[:], in_=x2sb[:])
                nc.sync.dma_start(out=dbg_xt[:], in_=xt_sb[:])
                nc.sync.dma_start(out=dbg_st[:], in_=st_big[:])
                nc.sync.dma_start(out=dbg_y[:], in_=ystage[:])
                nc.sync.dma_start(out=dbg_pg[:], in_=pg[:])

            # ---- step 3 (bf16): out = CS2 @ Y, K = 128 rows (s, c, t).
            # Quad [128, 2048] output tiles; copies alternate DVE/ACT; one
            # 512KB contiguous DMA per quad, alternating HWDGE rings.
            # ACT also issues the odd-quad out DMAs, so DVE takes 9 of each
            # 16 copies to balance engine time.
            act_copy = {1, 3, 5, 7, 9, 11, 13}
            for g in range(8):
                ot = outp.tile([128, 4 * 512], bf16, name="ot", tag="ot")
                for i in range(4):
                    m = g * 4 + i
                    ps = pacc.tile([128, 512], f32, name="ps3", tag="acc")
                    nc.tensor.matmul(
                        ps[:],
                        lhsT=cmat_sb[:, m * 128 : (m + 1) * 128],
                        rhs=pg[:],
                        start=True,
                        stop=True,
                    )
                    cp = (
                        nc.scalar.copy
                        if (m % 16) in act_copy
                        else nc.vector.tensor_copy
                    )
                    cp(ot[:, i * 512 : (i + 1) * 512], ps[:])
                oeng = nc.sync if g % 2 == 0 else nc.scalar
                oeng.dma_start(
                    out=out[:, g],
                    in_=ot[:].rearrange("p (i e) -> p i e", i=4),
                )

    nc.compile()
    return nc


_NC_CACHE = None


def _get_nc():
    global _NC_CACHE
    if _NC_CACHE is None:
        _NC_CACHE = build_nc()
    return _NC_CACHE


def _prep_w(w, sl):
    import ml_dtypes

    # [D, D, M] -> modes sl -> [128, T, 4, 512]: out[p, t, dc, e] = w[dc*128+p, e, t]
    wt = np.asarray(w)[:, :, sl]  # [d, e, T]
    wt = wt.reshape(4, 128, 512, T).transpose(1, 3, 0, 2)
    return np.ascontiguousarray(wt.astype(ml_dtypes.bfloat16))


def make_in_maps(q, w_real, w_imag):
    import ml_dtypes

    bf16 = ml_dtypes.bfloat16
    q = np.asarray(q)
    fmat_np, cmat_np, cmb_np = _constants()
    in_maps = []
    for c in range(NCORES):
        sl = slice(c * T, (c + 1) * T)
        qc = np.ascontiguousarray(
            q[c].astype(bf16).reshape(32, 128, 512).transpose(1, 0, 2)
        )  # [128, 32, 512]
        in_maps.append(
            {
                "qb": qc,
                "wr": _prep_w(w_real, sl),
                "wi": _prep_w(w_imag, sl),
                "fmat": fmat_np,
                "cmat": cmat_np,
                "cmb": cmb_np,
                "ident": np.eye(128, dtype=bf16),
            }
        )
    return in_maps


def gather_out(outs):
    full = []
    for o in outs:
        o = np.asarray(o)  # [128, 8, 4, 512]
        full.append(
            o.transpose(1, 2, 0, 3).reshape(L, D).astype(np.float32)
        )
    return np.stack(full, axis=0)


def run(q, w_real, w_imag, trace=False, debug=False, trace_cores=None):
    from concourse.bass_utils import run_bass_kernel_spmd

    nc = build_nc(debug=True) if debug else _get_nc()
    in_maps = make_in_maps(q, w_real, w_imag)
    res = run_bass_kernel_spmd(
        nc, in_maps, core_ids=list(range(NCORES)), trace=trace,
        trace_cores=trace_cores,
    )
    out = gather_out([r["out"] for r in res.results])
    return out, res


def kernel(q, w_real, w_imag):
    out, _ = run(q, w_real, w_imag)
    return out


# revision 18
# speedup vs baseline: 1.1149x; 1.1149x over previous
"""FourierBlock kernel for 8 TRN2 NeuronCores (v4).

Math: the reference keeps only the first 64 rfft modes, so the whole op is
    out[b] = CS2 @ Y[b],  Y = combine(P),  P = products(X2, W),  X2 = F2 @ q[b]
with F2 [128,4096] = [cos; -sin] forward-DFT rows, P the four uncombined
product groups (XrWr, XiWr, XrWi, XiWi), `combine` a [48,16] +-1 matmul
folding them into Yr = XrWr - XiWi / Yi = XrWi + XiWr, and CS2 [128, L] the
inverse-DFT rows (2/L scaling, DC halved).

Sharding: core c owns batch c for steps 1/3 (data parallel) and modes
[8c, 8c+8) for step 2 (tensor parallel over modes -> W is read exactly once
across the chip).  Two AllToAlls exchange the small X2 / Y tensors.

v4 changes vs v3 (trace-driven):
- Dummy 1KB AllToAll fired at t=0: the FIRST collective after load pays a
  40-100us ncfw arming cost (doorbell -> ALGO_MESH_BEGIN); absorbing it in a
  dummy overlaps the arming with the q/W prologue, so the real A2A1 starts
  within ~1us of its doorbell like A2A2 always did.
- Host-side contiguous layouts for q and out (no strided 1KB descriptors);
  out is un-shuffled on the host.
- Three DMA lanes: big streams (fmat halves, q, W, out) split across the two
  HWDGE rings (sync/scalar); small latency-critical transfers (b1in bounce,
  b2in staging, pg loads) on the idle GpSimd SWDGE ring so they never queue
  behind a 4MB W stream.
- xm transpose via 4 DMA-transposes (HWDGE xbar) directly b1out->xt_sb,
  removing the PE transposes + DVE copies and the xm_sb intermediate.
- Step 2 copies: one [48,512] PSUM->SBUF copy per mode (cmb has zero rows in
  the 16-31 band; the 3 PSUM bufs are memset once so the dead band is 0),
  alternating DVE/ACT, with the combine-output copy on the other engine.
- A2A2 split in two mode-halves: the first exchange flies while the second
  half's products run, so only one ~6us mesh is exposed.
- WARM2 enlarged to bridge the A2A2b window so step 3 runs at 2.4 GHz.
"""

import numpy as np

B, L, D, M = 8, 4096, 512, 64
NCORES = 8
T = M // NCORES  # local modes per core
TH = T // 2  # modes per A2A2 half

WARM1 = 48  # N=128 filler matmuls covering the A2A1 window
WARM2 = 150  # N=128 fillers bridging A2A2b + pg load before step 3


def _constants():
    import ml_dtypes

    bf16 = ml_dtypes.bfloat16
    k = np.arange(M)
    l = np.arange(L)
    ang = 2 * np.pi * np.outer(k, l) / L  # [M, L]
    # F2 row order (s, a, t): partition p = s*16 + a*8 + t holds
    # cos (a=0) / -sin (a=1) of mode k = 8s + t, so x2's partition layout
    # already equals the AllToAll bounce layout [s][a, t] (straight DMA).
    F2 = np.stack([np.cos(ang), -np.sin(ang)], axis=0)  # [2, M, L]
    F2 = F2.reshape(2, NCORES, T, L).transpose(1, 0, 2, 3).reshape(128, L)
    # lhsT chunks, p-major for contiguous DMA: fmat[p, n, m] = F2[m, n*128+p]
    fmat = np.ascontiguousarray(
        F2.T.reshape(32, 128, 128).transpose(1, 0, 2), dtype=bf16
    )  # [128, 32, 128] bf16

    # Inverse-DFT rows, K order (s, c, t) = mode 8s+t, c = Re/Im part of Y.
    ck = np.where(k == 0, 1.0, 2.0) / L
    ang2 = 2 * np.pi * np.outer(l, k) / L  # [L, M]
    C = (ck * np.cos(ang2)).reshape(L, NCORES, T)  # coeff for Yr
    S = (-(2.0 / L) * np.sin(ang2)).reshape(L, NCORES, T)  # coeff for Yi
    CS2 = np.empty((L, NCORES, 2, T))
    CS2[:, :, 0] = C
    CS2[:, :, 1] = S
    # K-row order (h, s, c, th) with t = h*TH + th, so each A2A2 half's Y
    # lands in a contiguous 64-partition range of pg.
    CS2 = CS2.reshape(L, NCORES, 2, 2, TH).transpose(0, 3, 1, 2, 4)
    cmat = np.ascontiguousarray(CS2.reshape(L, 128).T.astype(bf16))  # [128, L]

    # Producer-side combine: po rows (j, a) in band g2*32 -> Y rows (j, c).
    # Yr = XrWr - XiWi;  Yi = XrWi + XiWr.
    G = np.zeros((2, 2, 2))  # [c, a, g2]
    G[0, 0, 0] = 1.0
    G[0, 1, 1] = -1.0
    G[1, 0, 1] = 1.0
    G[1, 1, 0] = 1.0
    c48 = np.zeros((48, 16))
    for j in range(8):
        for a in range(2):
            for g2 in range(2):
                for c in range(2):
                    c48[g2 * 32 + j * 2 + a, j * 2 + c] = G[c, a, g2]
    c48 = np.ascontiguousarray(c48, dtype=bf16)
    return fmat, cmat, c48


def build_nc(reps=1, debug=False):
    import concourse.bacc as bacc
    import concourse.mybir as mybir
    import concourse.tile as tile

    f32 = mybir.dt.float32
    bf16 = mybir.dt.bfloat16
    nc = bacc.Bacc("TRN2", target_bir_lowering=False, num_devices=NCORES)

    # q pre-arranged on host: qb[p, n, d] = q[c][n*128 + p, d]
    qb = nc.dram_tensor("qb", [128, 32, D], bf16, kind="ExternalInput")
    # W pre-arranged on host: w[g2][p, t, dc, e] = W_g2[dc*128+p, e, 8c+t], bf16
    wr = nc.dram_tensor("wr", [128, T, 4, 512], bf16, kind="ExternalInput")
    wi = nc.dram_tensor("wi", [128, T, 4, 512], bf16, kind="ExternalInput")
    # out[p, g, i, e] = out_full[g*512 + i*128 + p, e]  (host un-shuffles)
    out = nc.dram_tensor("out", [128, 8, 4, D], bf16, kind="ExternalOutput")

    fmat_d = nc.dram_tensor("fmat", [128, 32, 128], bf16, kind="ExternalInput")
    cmat_d = nc.dram_tensor("cmat", [128, L], bf16, kind="ExternalInput")
    cmb_d = nc.dram_tensor("cmb", [48, 16], bf16, kind="ExternalInput")
    ident_d = nc.dram_tensor("ident", [128, 128], bf16, kind="ExternalInput")
    if debug:
        dbg_x2 = nc.dram_tensor("dbg_x2", [128, 512], bf16, kind="ExternalOutput")
        dbg_xt = nc.dram_tensor("dbg_xt", [128, 512], bf16, kind="ExternalOutput")
        dbg_st = nc.dram_tensor("dbg_st", [48, T * 512], bf16, kind="ExternalOutput")
        dbg_y = nc.dram_tensor("dbg_y", [16, T * 512], bf16, kind="ExternalOutput")
        dbg_pg = nc.dram_tensor("dbg_pg", [128, 512], bf16, kind="ExternalOutput")

    RG = [list(range(NCORES))]

    from concourse.tile_rust import add_dep_helper

    with tile.TileContext(nc) as tc:
        with (
            tc.tile_pool(name="constp", bufs=1) as constp,
            tc.tile_pool(name="qpool", bufs=5) as qpool,
            tc.tile_pool(name="wpool", bufs=1) as wpool,
            tc.tile_pool(name="misc", bufs=1) as misc,
            tc.tile_pool(name="outp", bufs=8) as outp,
            tc.tile_pool(name="pacc", bufs=4, space="PSUM") as pacc,
            tc.tile_pool(name="ptp", bufs=1, space="PSUM") as ptp,
            tc.tile_pool(name="po", bufs=3, space="PSUM") as po,
            tc.tile_pool(name="dram", bufs=1, space="DRAM") as dram,
        ):
          for rep in range(reps):
            # ---- dummy collective: absorbs the one-time ncfw arming cost
            # (40-100us doorbell->mesh-begin on the first collective) while
            # the prologue streams q/W.  Payload is 1KB of uninitialized
            # DRAM; the output is never read.
            d_in = dram.tile([NCORES, 64], bf16, name="d_in")
            d_out = dram.tile([NCORES, 64], bf16, name="d_out")
            nc.gpsimd.collective_compute(
                "AllToAll",
                mybir.AluOpType.bypass,
                replica_groups=RG,
                ins=[d_in.opt()],
                outs=[d_out.opt()],
            )

            # ---- memset the 3 product-PSUM bufs once: the 16-31 partition
            # band is never written by the product matmuls but IS read by the
            # single [48,512] staging copy; cmb zeros it in the combine, but
            # NaN * 0 = NaN, so it must hold finite data.
            po_init = []
            for i in range(3):
                pz = po.tile([48, 512], f32, name="po_t", tag="o")
                nc.vector.memset(pz[:], 0.0)
                po_init.append(pz)

            # ---- constants + q + W on the two HWDGE rings.
            # sync ring:   fmat half 0, q chunks 0/2/4/6, wr (2x2MB), xt
            #              transposes, out even quads
            # scalar ring: fmat half 1, q chunks 1/3/5/7, ident/cmb, wi
            #              (2x2MB), cmat, out odd quads
            fmat_sb = constp.tile([128, 32 * 128], bf16, name="fmat_sb")
            fmat_v = fmat_sb[:].rearrange("p (n m) -> p n m", n=32)
            nc.sync.dma_start(out=fmat_v[:, 0:16], in_=fmat_d[:, 0:16])

            # ---- step 1 (bf16): X2 = F2 @ qb -> [128 (s,a,t), 512 d].
            # DMA issue and matmuls interleaved per chunk (v3 order): this
            # keeps Tile's DMA-completion sem assignment local, so an early
            # matmul never waits on a lane whose count clears only when a
            # late W/cmat DMA lands.
            x2ps = pacc.tile([128, 512], f32, name="x2ps", tag="acc")
            q_dmas = []
            for lo in range(8):
                qt = qpool.tile([128, 4 * 512], bf16, name="qt", tag="qt")
                qeng = nc.sync if lo % 2 == 0 else nc.scalar
                q_dmas.append(qeng.dma_start(
                    out=qt[:].rearrange("p (n d) -> p n d", n=4),
                    in_=qb[:, lo * 4 : (lo + 1) * 4],
                ))
                if lo == 0:
                    # fmat half 1 early on scalar: chunks 16-31 need it
                    nc.scalar.dma_start(
                        out=fmat_v[:, 16:32], in_=fmat_d[:, 16:32]
                    )
                for li in range(4):
                    gl = lo * 4 + li
                    nc.tensor.matmul(
                        x2ps[:],
                        lhsT=fmat_sb[:, gl * 128 : (gl + 1) * 128],
                        rhs=qt[:, li * 512 : (li + 1) * 512],
                        start=(gl == 0),
                        stop=(gl == 31),
                    )
            cmb_sb = constp.tile([48, 16], bf16, name="cmb_sb")
            nc.scalar.dma_start(out=cmb_sb[:], in_=cmb_d[:])
            ident_sb = constp.tile([128, 128], bf16, name="ident_sb")
            nc.scalar.dma_start(out=ident_sb[:], in_=ident_d[:])

            # ---- W stream: 2x2MB per tensor per ring.  The first chunk of
            # each tensor is gated on the last q chunks: ring-FIFO only
            # orders descriptor GENERATION, the SDMA engines drain all
            # queued packets round-robin, so un-gated W data would steal
            # half of q's HBM bandwidth and starve step 1.
            w_sb = []
            for g2 in range(2):
                wt = wpool.tile([128, T * 4 * 512], bf16, name=f"w{g2}")
                w_sb.append(wt)
            for g2, wsrc in enumerate((wr, wi)):
                weng = nc.sync if g2 == 0 else nc.scalar
                for h in range(2):
                    wdma = weng.dma_start(
                        out=w_sb[g2][:, h * 8192 : (h + 1) * 8192].rearrange(
                            "p (t n e) -> p t n e", t=4, n=4
                        ),
                        in_=wsrc[:, h * 4 : (h + 1) * 4],
                    )
                    if h == 0:
                        for qd in q_dmas[-2:]:
                            add_dep_helper(
                                wdma.ins, qd.ins, sync=True,
                                reason="q keeps full HBM bandwidth",
                            )
            cmat_sb = constp.tile([128, L], bf16, name="cmat_sb")
            nc.scalar.dma_start(out=cmat_sb[:], in_=cmat_d[:])

            # ---- x2 PSUM -> SBUF (split DVE/ACT), bounce on SWDGE ring
            x2sb = misc.tile([128, 512], bf16, name="x2sb")
            nc.vector.tensor_copy(x2sb[:, 0:256], x2ps[:, 0:256])
            nc.scalar.copy(x2sb[:, 256:512], x2ps[:, 256:512])

            b1in = dram.tile([NCORES, 2, T, D], bf16, name="b1in")
            nc.gpsimd.dma_start(
                out=b1in[:].rearrange("s a t d -> (s a t) d"), in_=x2sb[:]
            )
            # Keep the PE clock warm through a short A2A1 stall (HAM
            # re-throttles to 1.2 GHz after ~3.4us idle).
            warm1 = pacc.tile([128, 512], f32, name="warm1", tag="acc")
            for i in range(WARM1):
                nc.tensor.matmul(
                    warm1[:, 0:128], lhsT=x2sb[:, 0:128], rhs=x2sb[:, 0:128],
                    start=(i == 0), stop=(i == WARM1 - 1),
                )
            b1out = dram.tile([NCORES, 2, T, D], bf16, name="b1out")
            nc.gpsimd.collective_compute(
                "AllToAll",
                mybir.AluOpType.bypass,
                replica_groups=RG,
                ins=[b1in.opt()],
                outs=[b1out.opt()],
            )

            # ---- load Xm [128 (j,a,t), 512 d] on the SWDGE ring, transpose
            # on the PE (re-warms it) into xt_sb[p=d|dc, (j a t)].
            xm_sb = misc.tile([128, 512], bf16, name="xm_sb")
            nc.gpsimd.dma_start(
                out=xm_sb[:], in_=b1out[:].rearrange("j a t d -> (j a t) d")
            )
            xt_sb = misc.tile([128, 512], bf16, name="xt_sb")
            for dc in range(4):
                tp = ptp.tile([128, 128], bf16, name="tp", tag="tp")
                nc.tensor.transpose(
                    tp[:], xm_sb[:, dc * 128 : (dc + 1) * 128], ident_sb[:]
                )
                cp = nc.vector.tensor_copy if dc % 2 else nc.scalar.copy
                cp(xt_sb[:, dc * 128 : (dc + 1) * 128], tp[:])

            # ---- step 2 (bf16) + producer-side combine: per t, both g2
            # product groups land in one [48, 512] PSUM tile (bands 0-15 /
            # 32-47), bounce through SBUF, and a [48,16] +-1 matmul folds
            # them into Y[(j, c)].  Modes are processed in two halves with
            # an AllToAll per half so the first exchange hides under the
            # second half's products.
            xt_v = xt_sb[:].rearrange("p (dc m t) -> p dc m t", dc=4, m=16, t=T)
            st_big = misc.tile([48, T * 512], bf16, name="st_big")
            b2in = [
                dram.tile([NCORES, 2, TH, D], bf16, name=f"b2in{h}")
                for h in range(2)
            ]
            b2out = [
                dram.tile([NCORES, 2, TH, D], bf16, name=f"b2out{h}")
                for h in range(2)
            ]
            ystage = misc.tile([16, T * 512], bf16, name="ystage")

            def emit_products(t):
                po_t = po.tile([48, 512], f32, name="po_t", tag="o")
                for g2 in range(2):
                    for dc in range(4):
                        nc.tensor.matmul(
                            po_t[g2 * 32 : g2 * 32 + 16, :],
                            lhsT=xt_v[:, dc, :, t],
                            rhs=w_sb[g2][
                                :, (t * 4 + dc) * 512 : (t * 4 + dc + 1) * 512
                            ],
                            start=(dc == 0),
                            stop=(dc == 3),
                        )
                sl = st_big[:, t * 512 : (t + 1) * 512]
                cp = nc.vector.tensor_copy if t % 2 else nc.scalar.copy
                cp(sl, po_t[:])
                return sl

            def emit_combine(t, sl):
                yt = po.tile([16, 512], f32, name="yt", tag="o")
                nc.tensor.matmul(
                    yt[:], lhsT=cmb_sb[:], rhs=sl, start=True, stop=True
                )
                ysl = ystage[:, t * 512 : (t + 1) * 512]
                cp = nc.scalar.copy if t % 2 else nc.vector.tensor_copy
                cp(ysl, yt[:])
                h, th = divmod(t, TH)
                nc.gpsimd.dma_start(
                    out=b2in[h][:, :, th].rearrange("j c e -> (j c) e"),
                    in_=ysl,
                )

            # Lag the combine one mode behind the products so the PE never
            # waits on the staging copies; fire each A2A2 half as soon as
            # its last mode is staged.
            slots = {}
            for t in range(T):
                slots[t] = emit_products(t)
                if t >= 1:
                    emit_combine(t - 1, slots[t - 1])
                if t - 1 == TH - 1:
                    nc.gpsimd.collective_compute(
                        "AllToAll",
                        mybir.AluOpType.bypass,
                        replica_groups=RG,
                        ins=[b2in[0].opt()],
                        outs=[b2out[0].opt()],
                    )
            emit_combine(T - 1, slots[T - 1])
            nc.gpsimd.collective_compute(
                "AllToAll",
                mybir.AluOpType.bypass,
                replica_groups=RG,
                ins=[b2in[1].opt()],
                outs=[b2out[1].opt()],
            )

            warm2 = pacc.tile([128, 512], f32, name="warm2", tag="acc")
            for i in range(WARM2):
                nc.tensor.matmul(
                    warm2[:, 0:128],
                    lhsT=ystage[:, 0:128],
                    rhs=ystage[:, 0:128],
                    start=(i == 0), stop=(i == WARM2 - 1),
                )

            # ---- load Y [128 (h,s,c,th), 512]: one SWDGE DMA per half into
            # a contiguous 64-partition range (cmat's K order matches).
            pg = misc.tile([128, 512], bf16, name="pg")
            for h in range(2):
                nc.gpsimd.dma_start(
                    out=pg[h * 64 : (h + 1) * 64],
                    in_=b2out[h][:].rearrange("s c t e -> (s c t) e"),
                )

            if debug:
                nc.sync.dma_start(out=dbg_x2[:], in_=x2sb[:])
                nc.sync.dma_start(out=dbg_xt[:], in_=xt_sb[:])
                nc.sync.dma_start(out=dbg_st[:], in_=st_big[:])
                nc.sync.dma_start(out=dbg_y[:], in_=ystage[:])
                nc.sync.dma_start(out=dbg_pg[:], in_=pg[:])

            # ---- step 3 (bf16): out = CS2 @ Y, K = 128 rows (s, c, t).
            # Quad [128, 2048] output tiles; copies alternate DVE/ACT; one
            # 512KB contiguous DMA per quad, alternating HWDGE rings.
            # ACT also issues the odd-quad out DMAs, so DVE takes 9 of each
            # 16 copies to balance engine time.
            act_copy = {1, 3, 5, 7, 9, 11, 13}
            for g in range(8):
                ot = outp.tile([128, 4 * 512], bf16, name="ot", tag="ot")
                for i in range(4):
                    m = g * 4 + i
                    ps = pacc.tile([128, 512], f32, name="ps3", tag="acc")
                    nc.tensor.matmul(
                        ps[:],
                        lhsT=cmat_sb[:, m * 128 : (m + 1) * 128],
                        rhs=pg[:],
                        start=True,
                        stop=True,
                    )
                    cp = (
                        nc.scalar.copy
                        if (m % 16) in act_copy
                        else nc.vector.tensor_copy
                    )
                    cp(ot[:, i * 512 : (i + 1) * 512], ps[:])
                oeng = nc.sync if g % 2 == 0 else nc.scalar
                oeng.dma_start(
                    out=out[:, g],
                    in_=ot[:].rearrange("p (i e) -> p i e", i=4),
                )

    nc.compile()
    return nc


_NC_CACHE = None


def _get_nc():
    global _NC_CACHE
    if _NC_CACHE is None:
        _NC_CACHE = build_nc()
    return _NC_CACHE


def _prep_w(w, sl):
    import ml_dtypes

    # [D, D, M] -> modes sl -> [128, T, 4, 512]: out[p, t, dc, e] = w[dc*128+p, e, t]
    wt = np.asarray(w)[:, :, sl]  # [d, e, T]
    wt = wt.reshape(4, 128, 512, T).transpose(1, 3, 0, 2)
    return np.ascontiguousarray(wt.astype(ml_dtypes.bfloat16))


def make_in_maps(q, w_real, w_imag):
    import ml_dtypes

    bf16 = ml_dtypes.bfloat16
    q = np.asarray(q)
    fmat_np, cmat_np, cmb_np = _constants()
    in_maps = []
    for c in range(NCORES):
        sl = slice(c * T, (c + 1) * T)
        qc = np.ascontiguousarray(
            q[c].astype(bf16).reshape(32, 128, 512).transpose(1, 0, 2)
        )  # [128, 32, 512]
        in_maps.append(
            {
                "qb": qc,
                "wr": _prep_w(w_real, sl),
                "wi": _prep_w(w_imag, sl),
                "fmat": fmat_np,
                "cmat": cmat_np,
                "cmb": cmb_np,
                "ident": np.eye(128, dtype=bf16),
            }
        )
    return in_maps


def gather_out(outs):
    full = []
    for o in outs:
        o = np.asarray(o)  # [128, 8, 4, 512]
        full.append(
            o.transpose(1, 2, 0, 3).reshape(L, D).astype(np.float32)
        )
    return np.stack(full, axis=0)


def run(q, w_real, w_imag, trace=False, debug=False, trace_cores=None):
    from concourse.bass_utils import run_bass_kernel_spmd

    nc = build_nc(debug=True) if debug else _get_nc()
    in_maps = make_in_maps(q, w_real, w_imag)
    res = run_bass_kernel_spmd(
        nc, in_maps, core_ids=list(range(NCORES)), trace=trace,
        trace_cores=trace_cores,
    )
    out = gather_out([r["out"] for r in res.results])
    return out, res


def kernel(q, w_real, w_imag):
    out, _ = run(q, w_real, w_imag)
    return out
